# revision 17
# baseline (speedup 1.0000x reference)
"""CLAM hierarchical attention-MIL kernel for 8 Trainium2 NeuronCores.

Reference structure: 4 levels of gated-attention scoring with top-p/radius
selection between levels (1000 -> ~2400 -> ~1000 -> ~100 rows), then softmax
attention pooling + a tiny classifier.

Device/host split (selection is discrete -- the output SHAPE depends on it --
so score matmuls are fp32: bf16 noise ~1e-3 exceeds adjacent-score gaps near
the k-th boundary, fp32 PE noise ~1e-6 does not):

  Launch A (8-way row-sharded, 2 segments): fp32 attention scores for ALL
      rows of h3 (level 0) and h2 (level 1).  Level-1 scores are only needed
      on the level-0-selected subset, but that subset is unknown before the
      launch and scores are row-independent, so we compute all 4000.
  Host: exact fp32 replication of the reference's top-k + radius-union
      selection.  After launch A both sel2 (h2 subset) and sel1 (exact h1
      subset, 1012 rows) are known, plus over0 = radius union over ALL sel1
      centers, a guaranteed superset (~375 rows) of the final h0 selection.
  Launch B (uniform program "R rows x one param set"): a load-balanced core
      split scores h1[sel1] with level-2 params on most cores while the rest
      run h0[over0] with level-3 params, also emitting the relu hidden x for
      pooling.
  Host: final top-k -> sel0 (subset of over0), softmax pooling, classifier.
"""

import numpy as np

EMBED = 1024
H1 = 512
H2 = 256
TOPP = 0.3
WIN = 225.0
SF = 4.0
OFF = 84.0  # 224/2 - 224/4/2
R2 = np.float32((WIN * SF) ** 2)  # 810000.0, exact in fp32

N_CORES = 8
GMAX = 512  # matmul moving-operand free dim (one fp32 PSUM bank)

# launch A: per-core padded row counts for (h3, h2)
A_LEVEL_ROWS = [1000, 4000]
A_CORE_ROWS = [128, 512]

# Populated on every kernel() call; read by test harnesses.
LAST_RUN_INFO = {}

_CACHE = {}


# ---------------------------------------------------------------- device code


def _emit_level(nc, tc, pools, hT_dram, g_total, w_dram, out_a, out_xT=None, gmax=GMAX,
                wab_early=False, w1_pieces=1):
    """Gated-attention scoring over g_total rows for one parameter set.

    hT_dram: [EMBED, g_total] fp32 (features x rows, this core's shard)
    out_a:   [1, g_total] fp32 -- (a*g)@Wc scores (bias bc added on host)
    out_xT:  optional [128, 4, g_total] fp32, slot (p, m, g) = x[g, m*128+p]
    """
    import concourse.mybir as mybir

    F = mybir.ActivationFunctionType
    ALU = mybir.AluOpType
    wpool, hpool, xpool, agpool, apool, ps, psA = pools

    KC1 = EMBED // 128  # 8 contraction chunks for W1
    M1 = H1 // 128      # 4 output tiles of x
    KC2 = H1 // 128     # 4 contraction chunks for Wa/Wb
    M2 = H2 // 128      # 2 output tiles of a/g
    KC3 = H2 // 128     # 2 contraction chunks for Wc

    dt = mybir.dt.float32

    # ---- weights into SBUF.  DMA issue is serialized (~1.2us fixed per
    # dma_start on SP.SEQ + HWDGE in the cost model), so order by when the
    # compute needs the data (BP + W1-chunk0 + group0-hT-chunk0 unblock the
    # first matmul+relu) and batch everything else into few large DMAs.
    # BP packs [b1(4) | ba(2) | bb(2) | Wc(2)] as 10 per-partition columns;
    # WAB packs [Wa | Wb] along the output-feature axis.
    groups = [
        (g0, min(gmax, g_total - g0)) for g0 in range(0, g_total, gmax)
    ]
    g00, G0 = groups[0]
    w1 = wpool.tile([128, KC1, H1], dt, tag="w1", name="w1")
    w1_src = w_dram["W1"].rearrange("(c p) m -> p c m", p=128)
    hT0 = hpool.tile([128, KC1, gmax], dt, tag="hT", name="hT")[:, :, :G0]
    bp = wpool.tile([128, 10], dt, tag="bp", name="bp")
    wab = wpool.tile([128, KC2, 2 * H2], dt, tag="wab", name="wab")
    wab_src = w_dram["WAB"].rearrange("(c p) m -> p c m", p=128)
    nc.sync.dma_start(bp[:], w_dram["BP"][:])
    nc.sync.dma_start(w1[:, 0, :], w1_src[:, 0, :])
    nc.sync.dma_start(hT0[:, 0, :], hT_dram[0:128, g00 : g00 + G0])
    bounds = [1] + [1 + (KC1 - 1) * (i + 1) // w1_pieces for i in range(w1_pieces)]
    for i in range(w1_pieces):
        lo, hi = bounds[i], bounds[i + 1]
        if lo < hi:
            nc.sync.dma_start(w1[:, lo:hi, :], w1_src[:, lo:hi, :])
    nc.sync.dma_start(
        hT0[:, 1:, :],
        hT_dram[128:EMBED, g00 : g00 + G0].rearrange("(c p) g -> p c g", p=128),
    )
    if wab_early:
        nc.sync.dma_start(wab[:], wab_src[:])
    hts = [hT0]
    for g0, G in groups[1:]:
        hTn = hpool.tile([128, KC1, gmax], dt, tag="hT", name="hT")[:, :, :G]
        nc.sync.dma_start(
            hTn[:, :, :],
            hT_dram[:, g0 : g0 + G].rearrange("(c p) g -> p c g", p=128),
        )
        hts.append(hTn)
    if not wab_early:
        nc.sync.dma_start(wab[:], wab_src[:])
    b1 = bp[:, 0:M1]
    ba = bp[:, M1 : M1 + M2]
    bb = bp[:, M1 + M2 : M1 + 2 * M2]
    wc = bp[:, M1 + 2 * M2 : M1 + 2 * M2 + KC3]

    for gi, (g0, G) in enumerate(groups):
        hT = hts[gi]

        # x^T = relu(W1^T h^T + b1): 4 tiles of [128, G].  relu on DVE keeps
        # ACT on a single table set (tanh+sigmoid).
        xT = xpool.tile([128, M1, gmax], dt, tag="xT", name="xT")[:, :, :G]
        for m in range(M1):
            acc = ps.tile([128, gmax], dt, tag="ps", name="ps")[:, :G]
            for c in range(KC1):
                nc.tensor.matmul(
                    acc,
                    w1[:, c, m * 128 : (m + 1) * 128],
                    hT[:, c, :],
                    start=(c == 0),
                    stop=(c == KC1 - 1),
                )
            nc.vector.tensor_scalar(
                xT[:, m, :], acc, b1[:, m : m + 1], 0.0, op0=ALU.add, op1=ALU.max
            )

        # a^T = tanh(Wa^T x^T + ba), g^T = sigmoid(Wb^T x^T + bb), ag = a*g
        agT = agpool.tile([128, M2, gmax], dt, tag="agT", name="agT")[:, :, :G]
        for m in range(M2):
            acc_a = ps.tile([128, gmax], dt, tag="ps", name="ps")[:, :G]
            for c in range(KC2):
                nc.tensor.matmul(
                    acc_a,
                    wab[:, c, m * 128 : (m + 1) * 128],
                    xT[:, c, :],
                    start=(c == 0),
                    stop=(c == KC2 - 1),
                )
            aT = agpool.tile([128, gmax], dt, tag="aT", name="aT")[:, :G]
            nc.scalar.activation(aT, acc_a, F.Tanh, bias=ba[:, m : m + 1])

            acc_g = ps.tile([128, gmax], dt, tag="ps", name="ps")[:, :G]
            for c in range(KC2):
                nc.tensor.matmul(
                    acc_g,
                    wab[:, c, 2 * H2 // 2 + m * 128 : 2 * H2 // 2 + (m + 1) * 128],
                    xT[:, c, :],
                    start=(c == 0),
                    stop=(c == KC2 - 1),
                )
            gT = agpool.tile([128, gmax], dt, tag="gT", name="gT")[:, :G]
            nc.scalar.activation(gT, acc_g, F.Sigmoid, bias=bb[:, m : m + 1])
            nc.vector.tensor_mul(agT[:, m, :], aT, gT)

        # A = Wc-contraction of (a*g) -> [1, G]
        acc_s = psA.tile([1, gmax], dt, tag="psA", name="psA")[:, :G]
        for c in range(KC3):
            nc.tensor.matmul(
                acc_s,
                wc[:, c : c + 1],
                agT[:, c, :],
                start=(c == 0),
                stop=(c == KC3 - 1),
            )
        a_st = apool.tile([1, gmax], dt, tag="a_st", name="a_st")[:, :G]
        nc.vector.tensor_copy(a_st, acc_s)
        nc.sync.dma_start(out_a[0:1, g0 : g0 + G], a_st)

        if out_xT is not None:
            nc.sync.dma_start(out_xT[gi, :, :, :G], xT)


def _level_dram_params(nc, tag):
    import concourse.mybir as mybir

    dt = mybir.dt.float32
    return {
        "W1": nc.dram_tensor(f"W1_{tag}", [EMBED, H1], dt, kind="ExternalInput"),
        "WAB": nc.dram_tensor(f"WAB_{tag}", [H1, 2 * H2], dt, kind="ExternalInput"),
        "BP": nc.dram_tensor(f"BP_{tag}", [128, 10], dt, kind="ExternalInput"),
    }


def _make_pools(tc, ctx):
    wpool = ctx.enter_context(tc.tile_pool(name="weights", bufs=2))
    hpool = ctx.enter_context(tc.tile_pool(name="hin", bufs=3))
    xpool = ctx.enter_context(tc.tile_pool(name="x", bufs=2))
    agpool = ctx.enter_context(tc.tile_pool(name="ag", bufs=2))
    apool = ctx.enter_context(tc.tile_pool(name="aout", bufs=3))
    ps = ctx.enter_context(tc.tile_pool(name="ps", bufs=6, space="PSUM"))
    psA = ctx.enter_context(tc.tile_pool(name="psA", bufs=2, space="PSUM"))
    return wpool, hpool, xpool, agpool, apool, ps, psA


def _build_stage_a():
    import concourse.bacc as bacc
    import concourse.tile as tile
    import concourse.mybir as mybir
    from contextlib import ExitStack

    dt = mybir.dt.float32
    nc = bacc.Bacc(None, target_bir_lowering=False)
    hT_drams = [
        nc.dram_tensor(f"hT{l}", [EMBED, A_CORE_ROWS[l]], dt, kind="ExternalInput")
        for l in range(2)
    ]
    w_drams = [_level_dram_params(nc, str(l)) for l in range(2)]
    out_drams = [
        nc.dram_tensor(f"a{l}", [1, A_CORE_ROWS[l]], dt, kind="ExternalOutput")
        for l in range(2)
    ]
    with tile.TileContext(nc) as tc:
        with ExitStack() as ctx:
            pools = _make_pools(tc, ctx)
            for l in (1, 0):
                _emit_level(
                    nc, tc, pools, hT_drams[l], A_CORE_ROWS[l], w_drams[l],
                    out_drams[l], gmax=256,
                )
    nc.compile()
    return nc


def _build_stage_b(rows):
    """Uniform program: score `rows` rows against ONE param set, emitting
    scores and the relu hidden xT.  Params/rows differ per core via in_maps."""
    import concourse.bacc as bacc
    import concourse.tile as tile
    import concourse.mybir as mybir
    from contextlib import ExitStack

    dt = mybir.dt.float32
    nc = bacc.Bacc(None, target_bir_lowering=False)
    gmax = min(GMAX, max(32, -(-rows // 3)))
    n_groups = -(-rows // gmax)
    hT_dram = nc.dram_tensor("hT", [EMBED, rows], dt, kind="ExternalInput")
    w_dram = _level_dram_params(nc, "w")
    out_a = nc.dram_tensor("a", [1, rows], dt, kind="ExternalOutput")
    out_xT = nc.dram_tensor(
        "xT", [n_groups, 128, H1 // 128, gmax], dt, kind="ExternalOutput"
    )
    with tile.TileContext(nc) as tc:
        with ExitStack() as ctx:
            pools = _make_pools(tc, ctx)
            # PE warm-up: this launch is short, so without it the PE spends
            # most of the kernel below max p-state.  A burst of matmuls on
            # zeroed SBUF bridges the initial DMA window and completes the
            # ~3us continuous-busy ramp before the real matmuls arrive.
            wp = ctx.enter_context(tc.tile_pool(name="warm", bufs=1))
            wdum = wp.tile([128, 128], dt, name="wdum")
            hdum = wp.tile([128, 256], dt, name="hdum")
            nc.gpsimd.memset(wdum[:], 0.0)
            nc.gpsimd.memset(hdum[:], 0.0)
            pdum = pools[5].tile([128, 256], dt, tag="ps", name="pdum")
            for _ in range(8):
                nc.tensor.matmul(pdum[:], wdum[:], hdum[:], start=True, stop=True)
            _emit_level(
                nc, tc, pools, hT_dram, rows, w_dram, out_a, out_xT=out_xT,
                gmax=gmax, wab_early=True, w1_pieces=3,
            )
    nc.compile()
    return nc


# ------------------------------------------------------------------ host code


def _np32(x):
    a = np.asarray(x)
    return a.astype(np.float32) if a.dtype != np.float32 else a


def _pad_rows(h, rows):
    if h.shape[0] == rows:
        return np.ascontiguousarray(h)
    out = np.zeros((rows, h.shape[1]), dtype=h.dtype)
    out[: h.shape[0]] = h
    return out


def _shard_hT(h, core_rows, n_shards):
    """h [N, EMBED] -> n_shards contiguous [EMBED, core_rows] fp32 shards."""
    hp = _pad_rows(_np32(h), core_rows * n_shards)
    hT = np.ascontiguousarray(hp.T)  # [EMBED, padded]
    return [
        np.ascontiguousarray(hT[:, c * core_rows : (c + 1) * core_rows])
        for c in range(n_shards)
    ]


def _level_param_inputs(tag, lp):
    bp = np.zeros((128, 10), dtype=np.float32)
    bp[:, 0:4] = _np32(lp["b1"]).reshape(4, 128).T
    bp[:, 4:6] = _np32(lp["ba"]).reshape(2, 128).T
    bp[:, 6:8] = _np32(lp["bb"]).reshape(2, 128).T
    bp[:, 8:10] = _np32(lp["Wc"]).reshape(-1).reshape(2, 128).T
    return {
        f"W1_{tag}": _np32(lp["W1"]),
        f"WAB_{tag}": np.ascontiguousarray(
            np.concatenate([_np32(lp["Wa"]), _np32(lp["Wb"])], axis=1)
        ),
        f"BP_{tag}": bp,
    }


def _topk_centers(a_scores, c_src, k):
    idx = np.argpartition(-a_scores, k - 1)[:k]
    return (c_src[idx] + np.float32(OFF)) * np.float32(SF)


def _radius_union(centers, c_tgt):
    """Exact fp32 replication of the reference's distance test; ascending idx."""
    d = c_tgt[None, :, :] - centers[:, None, :]
    d2 = d[..., 0] * d[..., 0] + d[..., 1] * d[..., 1]
    return np.nonzero((d2 < R2).any(axis=0))[0]


_NEFF_CACHE_DIR = "/var/tmp/bass_neff_cache"


def _install_neff_cache():
    """Memoize walrus NEFF compiles on disk, keyed by the BIR json hash.
    The backend compile (walrus + birsim) costs minutes per NEFF; the BIR
    emitted for a given build is deterministic, so a fresh process can reuse
    a previously compiled NEFF byte-for-byte."""
    import concourse.bass_utils as bu
    import concourse.bass2jax as b2j

    if getattr(bu, "_ant_neff_cache_installed", False):
        return
    import hashlib
    import os
    import shutil

    orig = bu.compile_bir_kernel

    def cached(bir_json, tmpdir, neff_name="file.neff"):
        try:
            os.makedirs(_NEFF_CACHE_DIR, exist_ok=True)
            raw = bir_json if isinstance(bir_json, bytes) else bir_json.encode()
            h = hashlib.sha256(raw).hexdigest()
            cpath = os.path.join(_NEFF_CACHE_DIR, f"{h}.neff")
            out = os.path.join(tmpdir, neff_name)
            if os.path.exists(cpath):
                shutil.copyfile(cpath, out)
                return out
        except Exception:
            return orig(bir_json, tmpdir, neff_name)
        res = orig(bir_json, tmpdir, neff_name)
        try:
            shutil.copyfile(res, cpath + f".tmp{os.getpid()}")
            os.replace(cpath + f".tmp{os.getpid()}", cpath)
        except Exception:
            pass
        return res

    bu.compile_bir_kernel = cached
    b2j.compile_bir_kernel = cached
    bu._ant_neff_cache_installed = True


def _run_spmd(nc, in_maps):
    import jax
    from concourse.bass_utils import run_bass_kernel_spmd

    _install_neff_cache()
    try:
        if not jax.config.jax_compilation_cache_dir:
            jax.config.update("jax_compilation_cache_dir", "/var/tmp/jax_pjrt_cache")
            jax.config.update("jax_persistent_cache_min_compile_time_secs", 0.0)
            jax.config.update("jax_persistent_cache_min_entry_size_bytes", 0)
    except Exception:
        pass
    return run_bass_kernel_spmd(nc, in_maps, core_ids=list(range(N_CORES)))


def kernel(h0, h1, h2, h3, coords0, coords1, coords2, coords3, params):
    h3, h2, h1, h0 = _np32(h3), _np32(h2), _np32(h1), _np32(h0)
    c3, c2, c1, c0 = map(_np32, (coords3, coords2, coords1, coords0))
    levels = params["levels"]

    # ---------------- launch A: scores for all rows of h3, h2
    if "A" not in _CACHE:
        _CACHE["A"] = _build_stage_a()
    nc_a = _CACHE["A"]

    shards = [
        _shard_hT(h3, A_CORE_ROWS[0], N_CORES),
        _shard_hT(h2, A_CORE_ROWS[1], N_CORES),
    ]
    wmaps = {}
    for l in range(2):
        wmaps.update(_level_param_inputs(str(l), levels[l]))
    in_maps = []
    for c in range(N_CORES):
        m = {f"hT{l}": shards[l][c] for l in range(2)}
        m.update(wmaps)
        in_maps.append(m)
    res_a = _run_spmd(nc_a, in_maps)
    A3 = np.concatenate([res_a.results[c]["a0"][0] for c in range(N_CORES)])[
        : A_LEVEL_ROWS[0]
    ]
    A2 = np.concatenate([res_a.results[c]["a1"][0] for c in range(N_CORES)])[
        : A_LEVEL_ROWS[1]
    ]

    # ---------------- host: selection through level 1 + over-approx for h0
    k0 = max(1, int(TOPP * A3.shape[0]))
    sel2 = _radius_union(_topk_centers(A3, c3, k0), c2)

    A2s = A2[sel2]
    k1 = max(1, int(TOPP * A2s.shape[0]))
    sel1 = _radius_union(_topk_centers(A2s, c2[sel2], k1), c1)

    # superset of the final h0 selection: union over ALL sel1 centers
    over0 = _radius_union((c1[sel1] + np.float32(OFF)) * np.float32(SF), c0)

    # ---------------- launch B: h1[sel1] on cores 0..c1n-1, h0[over0] on rest
    n1, n0 = max(1, int(sel1.shape[0])), max(1, int(over0.shape[0]))
    c1n = min(N_CORES - 1, max(1, round(N_CORES * n1 / (n1 + n0))))
    c0n = N_CORES - c1n
    rows = max(-(-n1 // c1n), -(-n0 // c0n))
    key = ("B", rows)
    if key not in _CACHE:
        _CACHE[key] = _build_stage_b(rows)
    nc_b = _CACHE[key]

    sh1 = _shard_hT(h1[sel1], rows, c1n)
    sh0 = _shard_hT(h0[over0], rows, c0n)
    p2 = _level_param_inputs("w", levels[2])
    p3 = _level_param_inputs("w", levels[3])
    in_maps_b = []
    for c in range(N_CORES):
        if c < c1n:
            m = {"hT": sh1[c]}
            m.update(p2)
        else:
            m = {"hT": sh0[c - c1n]}
            m.update(p3)
        in_maps_b.append(m)
    res_b = _run_spmd(nc_b, in_maps_b)

    A1s = np.concatenate([res_b.results[c]["a"][0] for c in range(c1n)])[:n1]
    A0o = np.concatenate([res_b.results[c1n + c]["a"][0] for c in range(c0n)])[:n0]
    def _decode_xT(arr):
        # arr [n_groups, 128, 4, gmax]; row g of group gi -> x[gi*gmax+g, m*128+p]
        ng, _, m4, gm = arr.shape
        x = arr.transpose(0, 3, 2, 1).reshape(ng * gm, m4 * 128)
        return x[:rows]

    x0o = np.concatenate(
        [_decode_xT(res_b.results[c1n + c]["xT"]) for c in range(c0n)]
    )[:n0]

    # ---------------- host: final selection (sel0 is a subset of over0)
    k2 = max(1, int(TOPP * A1s.shape[0]))
    sel0 = _radius_union(_topk_centers(A1s, c1[sel1], k2), c0)
    pos = np.searchsorted(over0, sel0)
    assert np.array_equal(over0[pos], sel0), "over0 must be a superset of sel0"

    bc3 = _np32(levels[3]["bc"]).reshape(-1)[0]
    A_sel = A0o[pos] + bc3
    x = x0o[pos]

    # ---------------- host: softmax pooling + classifier
    am = A_sel.max()
    e = np.exp(A_sel - am)
    w = e / e.sum()
    M = w @ x  # [512]
    logits = (M @ _np32(params["Wcls"]) + _np32(params["bcls"]))[None, :]
    lm = logits.max(axis=1, keepdims=True)
    le = np.exp(logits - lm)
    Y_prob = le / le.sum(axis=1, keepdims=True)
    Y_hat = np.argmax(logits, axis=1).astype(np.int32)
    A_raw = A_sel[None, :].astype(np.float32)

    LAST_RUN_INFO.clear()
    LAST_RUN_INFO.update(
        {
            "n_final": int(sel0.shape[0]),
            "rows_b": int(rows),
            "split": (c1n, c0n),
            "exec_time_ns": [res_a.exec_time_ns, res_b.exec_time_ns],
            "nc_a": nc_a,
            "nc_b": nc_b,
        }
    )
    return (
        logits.astype(np.float32),
        Y_prob.astype(np.float32),
        Y_hat,
        A_raw,
    )


# revision 18
# speedup vs baseline: 1.0129x; 1.0129x over previous
"""CLAM hierarchical attention-MIL kernel for 8 Trainium2 NeuronCores.

Reference structure: 4 levels of gated-attention scoring with top-p/radius
selection between levels (1000 -> ~2400 -> ~1000 -> ~100 rows), then softmax
attention pooling + a tiny classifier.

Device/host split (selection is discrete -- the output SHAPE depends on it --
so score matmuls are fp32: bf16 noise ~1e-3 exceeds adjacent-score gaps near
the k-th boundary, fp32 PE noise ~1e-6 does not):

  Launch A (8-way row-sharded, 2 segments): fp32 attention scores for ALL
      rows of h3 (level 0) and h2 (level 1).  Level-1 scores are only needed
      on the level-0-selected subset, but that subset is unknown before the
      launch and scores are row-independent, so we compute all 4000.
  Host: exact fp32 replication of the reference's top-k + radius-union
      selection.  After launch A both sel2 (h2 subset) and sel1 (exact h1
      subset, 1012 rows) are known, plus over0 = radius union over ALL sel1
      centers, a guaranteed superset (~375 rows) of the final h0 selection.
  Launch B (uniform program "R rows x one param set"): a load-balanced core
      split scores h1[sel1] with level-2 params on most cores while the rest
      run h0[over0] with level-3 params, also emitting the relu hidden x for
      pooling.
  Host: final top-k -> sel0 (subset of over0), softmax pooling, classifier.
"""

import numpy as np

EMBED = 1024
H1 = 512
H2 = 256
TOPP = 0.3
WIN = 225.0
SF = 4.0
OFF = 84.0  # 224/2 - 224/4/2
R2 = np.float32((WIN * SF) ** 2)  # 810000.0, exact in fp32

N_CORES = 8
GMAX = 512  # matmul moving-operand free dim (one fp32 PSUM bank)

# launch A: per-core row counts for (h3, h2) -- 1000 and 4000 divide by 8
# exactly, so shards carry no pad rows
A_LEVEL_ROWS = [1000, 4000]
A_CORE_ROWS = [125, 500]

# Populated on every kernel() call; read by test harnesses.
LAST_RUN_INFO = {}

_CACHE = {}


# ---------------------------------------------------------------- device code


def _emit_level(nc, tc, pools, hT_dram, g_total, w_dram, out_a, out_xT=None, gmax=GMAX,
                wab_early=False, w1_pieces=1):
    """Gated-attention scoring over g_total rows for one parameter set.

    hT_dram: [EMBED, g_total] fp32 (features x rows, this core's shard)
    out_a:   [1, g_total] fp32 -- (a*g)@Wc scores (bias bc added on host)
    out_xT:  optional [128, 4, g_total] fp32, slot (p, m, g) = x[g, m*128+p]
    """
    import concourse.mybir as mybir

    F = mybir.ActivationFunctionType
    ALU = mybir.AluOpType
    wpool, hpool, xpool, agpool, apool, ps, psA = pools

    KC1 = EMBED // 128  # 8 contraction chunks for W1
    M1 = H1 // 128      # 4 output tiles of x
    KC2 = H1 // 128     # 4 contraction chunks for Wa/Wb
    M2 = H2 // 128      # 2 output tiles of a/g
    KC3 = H2 // 128     # 2 contraction chunks for Wc

    dt = mybir.dt.float32

    # ---- weights into SBUF.  DMA issue is serialized (~1.2us fixed per
    # dma_start on SP.SEQ + HWDGE in the cost model), so order by when the
    # compute needs the data (BP + W1-chunk0 + group0-hT-chunk0 unblock the
    # first matmul+relu) and batch everything else into few large DMAs.
    # BP packs [b1(4) | ba(2) | bb(2) | Wc(2)] as 10 per-partition columns;
    # WAB packs [Wa | Wb] along the output-feature axis.
    groups = [
        (g0, min(gmax, g_total - g0)) for g0 in range(0, g_total, gmax)
    ]
    g00, G0 = groups[0]
    w1 = wpool.tile([128, KC1, H1], dt, tag="w1", name="w1")
    w1_src = w_dram["W1"].rearrange("(c p) m -> p c m", p=128)
    hT0 = hpool.tile([128, KC1, gmax], dt, tag="hT", name="hT")[:, :, :G0]
    bp = wpool.tile([128, 10], dt, tag="bp", name="bp")
    wab = wpool.tile([128, KC2, 2 * H2], dt, tag="wab", name="wab")
    wab_src = w_dram["WAB"].rearrange("(c p) m -> p c m", p=128)
    nc.sync.dma_start(bp[:], w_dram["BP"][:])
    nc.sync.dma_start(w1[:, 0, :], w1_src[:, 0, :])
    nc.sync.dma_start(hT0[:, 0, :], hT_dram[0:128, g00 : g00 + G0])
    bounds = [1] + [1 + (KC1 - 1) * (i + 1) // w1_pieces for i in range(w1_pieces)]
    for i in range(w1_pieces):
        lo, hi = bounds[i], bounds[i + 1]
        if lo < hi:
            nc.sync.dma_start(w1[:, lo:hi, :], w1_src[:, lo:hi, :])
    nc.sync.dma_start(
        hT0[:, 1:, :],
        hT_dram[128:EMBED, g00 : g00 + G0].rearrange("(c p) g -> p c g", p=128),
    )
    if wab_early:
        nc.sync.dma_start(wab[:], wab_src[:])
    hts = [hT0]
    for g0, G in groups[1:]:
        hTn = hpool.tile([128, KC1, gmax], dt, tag="hT", name="hT")[:, :, :G]
        nc.sync.dma_start(
            hTn[:, :, :],
            hT_dram[:, g0 : g0 + G].rearrange("(c p) g -> p c g", p=128),
        )
        hts.append(hTn)
    if not wab_early:
        nc.sync.dma_start(wab[:], wab_src[:])
    b1 = bp[:, 0:M1]
    ba = bp[:, M1 : M1 + M2]
    bb = bp[:, M1 + M2 : M1 + 2 * M2]
    wc = bp[:, M1 + 2 * M2 : M1 + 2 * M2 + KC3]

    for gi, (g0, G) in enumerate(groups):
        hT = hts[gi]

        # x^T = relu(W1^T h^T + b1): 4 tiles of [128, G].  relu on DVE keeps
        # ACT on a single table set (tanh+sigmoid).
        xT = xpool.tile([128, M1, gmax], dt, tag="xT", name="xT")[:, :, :G]
        for m in range(M1):
            acc = ps.tile([128, gmax], dt, tag="ps", name="ps")[:, :G]
            for c in range(KC1):
                nc.tensor.matmul(
                    acc,
                    w1[:, c, m * 128 : (m + 1) * 128],
                    hT[:, c, :],
                    start=(c == 0),
                    stop=(c == KC1 - 1),
                )
            nc.vector.tensor_scalar(
                xT[:, m, :], acc, b1[:, m : m + 1], 0.0, op0=ALU.add, op1=ALU.max
            )

        # a^T = tanh(Wa^T x^T + ba), g^T = sigmoid(Wb^T x^T + bb), ag = a*g
        agT = agpool.tile([128, M2, gmax], dt, tag="agT", name="agT")[:, :, :G]
        for m in range(M2):
            acc_a = ps.tile([128, gmax], dt, tag="ps", name="ps")[:, :G]
            for c in range(KC2):
                nc.tensor.matmul(
                    acc_a,
                    wab[:, c, m * 128 : (m + 1) * 128],
                    xT[:, c, :],
                    start=(c == 0),
                    stop=(c == KC2 - 1),
                )
            aT = agpool.tile([128, gmax], dt, tag="aT", name="aT")[:, :G]
            nc.scalar.activation(aT, acc_a, F.Tanh, bias=ba[:, m : m + 1])

            acc_g = ps.tile([128, gmax], dt, tag="ps", name="ps")[:, :G]
            for c in range(KC2):
                nc.tensor.matmul(
                    acc_g,
                    wab[:, c, 2 * H2 // 2 + m * 128 : 2 * H2 // 2 + (m + 1) * 128],
                    xT[:, c, :],
                    start=(c == 0),
                    stop=(c == KC2 - 1),
                )
            gT = agpool.tile([128, gmax], dt, tag="gT", name="gT")[:, :G]
            nc.scalar.activation(gT, acc_g, F.Sigmoid, bias=bb[:, m : m + 1])
            nc.vector.tensor_mul(agT[:, m, :], aT, gT)

        # A = Wc-contraction of (a*g) -> [1, G]
        acc_s = psA.tile([1, gmax], dt, tag="psA", name="psA")[:, :G]
        for c in range(KC3):
            nc.tensor.matmul(
                acc_s,
                wc[:, c : c + 1],
                agT[:, c, :],
                start=(c == 0),
                stop=(c == KC3 - 1),
            )
        a_st = apool.tile([1, gmax], dt, tag="a_st", name="a_st")[:, :G]
        nc.vector.tensor_copy(a_st, acc_s)
        nc.sync.dma_start(out_a[0:1, g0 : g0 + G], a_st)

        if out_xT is not None:
            nc.sync.dma_start(out_xT[gi, :, :, :G], xT)


def _level_dram_params(nc, tag):
    import concourse.mybir as mybir

    dt = mybir.dt.float32
    return {
        "W1": nc.dram_tensor(f"W1_{tag}", [EMBED, H1], dt, kind="ExternalInput"),
        "WAB": nc.dram_tensor(f"WAB_{tag}", [H1, 2 * H2], dt, kind="ExternalInput"),
        "BP": nc.dram_tensor(f"BP_{tag}", [128, 10], dt, kind="ExternalInput"),
    }


def _make_pools(tc, ctx):
    wpool = ctx.enter_context(tc.tile_pool(name="weights", bufs=2))
    hpool = ctx.enter_context(tc.tile_pool(name="hin", bufs=3))
    xpool = ctx.enter_context(tc.tile_pool(name="x", bufs=2))
    agpool = ctx.enter_context(tc.tile_pool(name="ag", bufs=2))
    apool = ctx.enter_context(tc.tile_pool(name="aout", bufs=3))
    ps = ctx.enter_context(tc.tile_pool(name="ps", bufs=6, space="PSUM"))
    psA = ctx.enter_context(tc.tile_pool(name="psA", bufs=2, space="PSUM"))
    return wpool, hpool, xpool, agpool, apool, ps, psA


def _build_stage_a():
    import concourse.bacc as bacc
    import concourse.tile as tile
    import concourse.mybir as mybir
    from contextlib import ExitStack

    dt = mybir.dt.float32
    nc = bacc.Bacc(None, target_bir_lowering=False)
    hT_drams = [
        nc.dram_tensor(f"hT{l}", [EMBED, A_CORE_ROWS[l]], dt, kind="ExternalInput")
        for l in range(2)
    ]
    w_drams = [_level_dram_params(nc, str(l)) for l in range(2)]
    out_drams = [
        nc.dram_tensor(f"a{l}", [1, A_CORE_ROWS[l]], dt, kind="ExternalOutput")
        for l in range(2)
    ]
    with tile.TileContext(nc) as tc:
        with ExitStack() as ctx:
            pools = _make_pools(tc, ctx)
            for l in (1, 0):
                _emit_level(
                    nc, tc, pools, hT_drams[l], A_CORE_ROWS[l], w_drams[l],
                    out_drams[l], gmax=250,
                )
    nc.compile()
    return nc


def _build_stage_b(rows):
    """Uniform program: score `rows` rows against ONE param set, emitting
    scores and the relu hidden xT.  Params/rows differ per core via in_maps."""
    import concourse.bacc as bacc
    import concourse.tile as tile
    import concourse.mybir as mybir
    from contextlib import ExitStack

    dt = mybir.dt.float32
    nc = bacc.Bacc(None, target_bir_lowering=False)
    gmax = min(GMAX, max(32, -(-rows // 3)))
    n_groups = -(-rows // gmax)
    hT_dram = nc.dram_tensor("hT", [EMBED, rows], dt, kind="ExternalInput")
    w_dram = _level_dram_params(nc, "w")
    out_a = nc.dram_tensor("a", [1, rows], dt, kind="ExternalOutput")
    out_xT = nc.dram_tensor(
        "xT", [n_groups, 128, H1 // 128, gmax], dt, kind="ExternalOutput"
    )
    with tile.TileContext(nc) as tc:
        with ExitStack() as ctx:
            pools = _make_pools(tc, ctx)
            # PE warm-up: this launch is short, so without it the PE spends
            # most of the kernel below max p-state.  A burst of matmuls on
            # zeroed SBUF bridges the initial DMA window and completes the
            # ~3us continuous-busy ramp before the real matmuls arrive.
            wp = ctx.enter_context(tc.tile_pool(name="warm", bufs=1))
            wdum = wp.tile([128, 128], dt, name="wdum")
            hdum = wp.tile([128, 256], dt, name="hdum")
            nc.gpsimd.memset(wdum[:], 0.0)
            nc.gpsimd.memset(hdum[:], 0.0)
            pdum = pools[5].tile([128, 256], dt, tag="ps", name="pdum")
            for _ in range(8):
                nc.tensor.matmul(pdum[:], wdum[:], hdum[:], start=True, stop=True)
            _emit_level(
                nc, tc, pools, hT_dram, rows, w_dram, out_a, out_xT=out_xT,
                gmax=gmax, wab_early=True, w1_pieces=3,
            )
    nc.compile()
    return nc


# ------------------------------------------------------------------ host code


def _np32(x):
    a = np.asarray(x)
    return a.astype(np.float32) if a.dtype != np.float32 else a


def _pad_rows(h, rows):
    if h.shape[0] == rows:
        return np.ascontiguousarray(h)
    out = np.zeros((rows, h.shape[1]), dtype=h.dtype)
    out[: h.shape[0]] = h
    return out


def _shard_hT(h, core_rows, n_shards):
    """h [N, EMBED] -> n_shards contiguous [EMBED, core_rows] fp32 shards."""
    hp = _pad_rows(_np32(h), core_rows * n_shards)
    hT = np.ascontiguousarray(hp.T)  # [EMBED, padded]
    return [
        np.ascontiguousarray(hT[:, c * core_rows : (c + 1) * core_rows])
        for c in range(n_shards)
    ]


def _level_param_inputs(tag, lp):
    bp = np.zeros((128, 10), dtype=np.float32)
    bp[:, 0:4] = _np32(lp["b1"]).reshape(4, 128).T
    bp[:, 4:6] = _np32(lp["ba"]).reshape(2, 128).T
    bp[:, 6:8] = _np32(lp["bb"]).reshape(2, 128).T
    bp[:, 8:10] = _np32(lp["Wc"]).reshape(-1).reshape(2, 128).T
    return {
        f"W1_{tag}": _np32(lp["W1"]),
        f"WAB_{tag}": np.ascontiguousarray(
            np.concatenate([_np32(lp["Wa"]), _np32(lp["Wb"])], axis=1)
        ),
        f"BP_{tag}": bp,
    }


def _topk_centers(a_scores, c_src, k):
    idx = np.argpartition(-a_scores, k - 1)[:k]
    return (c_src[idx] + np.float32(OFF)) * np.float32(SF)


def _radius_union(centers, c_tgt):
    """Exact fp32 replication of the reference's distance test; ascending idx."""
    d = c_tgt[None, :, :] - centers[:, None, :]
    d2 = d[..., 0] * d[..., 0] + d[..., 1] * d[..., 1]
    return np.nonzero((d2 < R2).any(axis=0))[0]


_NEFF_CACHE_DIR = "/var/tmp/bass_neff_cache"


def _install_neff_cache():
    """Memoize walrus NEFF compiles on disk, keyed by the BIR json hash.
    The backend compile (walrus + birsim) costs minutes per NEFF; the BIR
    emitted for a given build is deterministic, so a fresh process can reuse
    a previously compiled NEFF byte-for-byte."""
    import concourse.bass_utils as bu
    import concourse.bass2jax as b2j

    if getattr(bu, "_ant_neff_cache_installed", False):
        return
    import hashlib
    import os
    import shutil

    orig = bu.compile_bir_kernel

    def cached(bir_json, tmpdir, neff_name="file.neff"):
        try:
            os.makedirs(_NEFF_CACHE_DIR, exist_ok=True)
            raw = bir_json if isinstance(bir_json, bytes) else bir_json.encode()
            h = hashlib.sha256(raw).hexdigest()
            cpath = os.path.join(_NEFF_CACHE_DIR, f"{h}.neff")
            out = os.path.join(tmpdir, neff_name)
            if os.path.exists(cpath):
                shutil.copyfile(cpath, out)
                return out
        except Exception:
            return orig(bir_json, tmpdir, neff_name)
        res = orig(bir_json, tmpdir, neff_name)
        try:
            shutil.copyfile(res, cpath + f".tmp{os.getpid()}")
            os.replace(cpath + f".tmp{os.getpid()}", cpath)
        except Exception:
            pass
        return res

    bu.compile_bir_kernel = cached
    b2j.compile_bir_kernel = cached
    bu._ant_neff_cache_installed = True


def _run_spmd(nc, in_maps):
    import jax
    from concourse.bass_utils import run_bass_kernel_spmd

    _install_neff_cache()
    try:
        if not jax.config.jax_compilation_cache_dir:
            jax.config.update("jax_compilation_cache_dir", "/var/tmp/jax_pjrt_cache")
            jax.config.update("jax_persistent_cache_min_compile_time_secs", 0.0)
            jax.config.update("jax_persistent_cache_min_entry_size_bytes", 0)
    except Exception:
        pass
    return run_bass_kernel_spmd(nc, in_maps, core_ids=list(range(N_CORES)))


def kernel(h0, h1, h2, h3, coords0, coords1, coords2, coords3, params):
    h3, h2, h1, h0 = _np32(h3), _np32(h2), _np32(h1), _np32(h0)
    c3, c2, c1, c0 = map(_np32, (coords3, coords2, coords1, coords0))
    levels = params["levels"]

    # ---------------- launch A: scores for all rows of h3, h2
    if "A" not in _CACHE:
        _CACHE["A"] = _build_stage_a()
    nc_a = _CACHE["A"]

    shards = [
        _shard_hT(h3, A_CORE_ROWS[0], N_CORES),
        _shard_hT(h2, A_CORE_ROWS[1], N_CORES),
    ]
    wmaps = {}
    for l in range(2):
        wmaps.update(_level_param_inputs(str(l), levels[l]))
    in_maps = []
    for c in range(N_CORES):
        m = {f"hT{l}": shards[l][c] for l in range(2)}
        m.update(wmaps)
        in_maps.append(m)
    res_a = _run_spmd(nc_a, in_maps)
    A3 = np.concatenate([res_a.results[c]["a0"][0] for c in range(N_CORES)])[
        : A_LEVEL_ROWS[0]
    ]
    A2 = np.concatenate([res_a.results[c]["a1"][0] for c in range(N_CORES)])[
        : A_LEVEL_ROWS[1]
    ]

    # ---------------- host: selection through level 1 + over-approx for h0
    k0 = max(1, int(TOPP * A3.shape[0]))
    sel2 = _radius_union(_topk_centers(A3, c3, k0), c2)

    A2s = A2[sel2]
    k1 = max(1, int(TOPP * A2s.shape[0]))
    sel1 = _radius_union(_topk_centers(A2s, c2[sel2], k1), c1)

    # superset of the final h0 selection: union over ALL sel1 centers
    over0 = _radius_union((c1[sel1] + np.float32(OFF)) * np.float32(SF), c0)

    # ---------------- launch B: h1[sel1] on cores 0..c1n-1, h0[over0] on rest
    n1, n0 = max(1, int(sel1.shape[0])), max(1, int(over0.shape[0]))
    c1n = min(N_CORES - 1, max(1, round(N_CORES * n1 / (n1 + n0))))
    c0n = N_CORES - c1n
    rows = max(-(-n1 // c1n), -(-n0 // c0n))
    key = ("B", rows)
    if key not in _CACHE:
        _CACHE[key] = _build_stage_b(rows)
    nc_b = _CACHE[key]

    sh1 = _shard_hT(h1[sel1], rows, c1n)
    sh0 = _shard_hT(h0[over0], rows, c0n)
    p2 = _level_param_inputs("w", levels[2])
    p3 = _level_param_inputs("w", levels[3])
    in_maps_b = []
    for c in range(N_CORES):
        if c < c1n:
            m = {"hT": sh1[c]}
            m.update(p2)
        else:
            m = {"hT": sh0[c - c1n]}
            m.update(p3)
        in_maps_b.append(m)
    res_b = _run_spmd(nc_b, in_maps_b)

    A1s = np.concatenate([res_b.results[c]["a"][0] for c in range(c1n)])[:n1]
    A0o = np.concatenate([res_b.results[c1n + c]["a"][0] for c in range(c0n)])[:n0]
    def _decode_xT(arr):
        # arr [n_groups, 128, 4, gmax]; row g of group gi -> x[gi*gmax+g, m*128+p]
        ng, _, m4, gm = arr.shape
        x = arr.transpose(0, 3, 2, 1).reshape(ng * gm, m4 * 128)
        return x[:rows]

    x0o = np.concatenate(
        [_decode_xT(res_b.results[c1n + c]["xT"]) for c in range(c0n)]
    )[:n0]

    # ---------------- host: final selection (sel0 is a subset of over0)
    k2 = max(1, int(TOPP * A1s.shape[0]))
    sel0 = _radius_union(_topk_centers(A1s, c1[sel1], k2), c0)
    pos = np.searchsorted(over0, sel0)
    assert np.array_equal(over0[pos], sel0), "over0 must be a superset of sel0"

    bc3 = _np32(levels[3]["bc"]).reshape(-1)[0]
    A_sel = A0o[pos] + bc3
    x = x0o[pos]

    # ---------------- host: softmax pooling + classifier
    am = A_sel.max()
    e = np.exp(A_sel - am)
    w = e / e.sum()
    M = w @ x  # [512]
    logits = (M @ _np32(params["Wcls"]) + _np32(params["bcls"]))[None, :]
    lm = logits.max(axis=1, keepdims=True)
    le = np.exp(logits - lm)
    Y_prob = le / le.sum(axis=1, keepdims=True)
    Y_hat = np.argmax(logits, axis=1).astype(np.int32)
    A_raw = A_sel[None, :].astype(np.float32)

    LAST_RUN_INFO.clear()
    LAST_RUN_INFO.update(
        {
            "n_final": int(sel0.shape[0]),
            "rows_b": int(rows),
            "split": (c1n, c0n),
            "exec_time_ns": [res_a.exec_time_ns, res_b.exec_time_ns],
            "nc_a": nc_a,
            "nc_b": nc_b,
        }
    )
    return (
        logits.astype(np.float32),
        Y_prob.astype(np.float32),
        Y_hat,
        A_raw,
    )


# revision 19
# speedup vs baseline: 1.0250x; 1.0119x over previous
"""CLAM hierarchical attention-MIL kernel for 8 Trainium2 NeuronCores.

Reference structure: 4 levels of gated-attention scoring with top-p/radius
selection between levels (1000 -> ~2400 -> ~1000 -> ~100 rows), then softmax
attention pooling + a tiny classifier.

Device/host split (selection is discrete -- the output SHAPE depends on it --
so score matmuls are fp32: bf16 noise ~1e-3 exceeds adjacent-score gaps near
the k-th boundary, fp32 PE noise ~1e-6 does not):

  Launch A (8-way row-sharded, 2 segments): fp32 attention scores for ALL
      rows of h3 (level 0) and h2 (level 1).  Level-1 scores are only needed
      on the level-0-selected subset, but that subset is unknown before the
      launch and scores are row-independent, so we compute all 4000.
  Host: exact fp32 replication of the reference's top-k + radius-union
      selection.  After launch A both sel2 (h2 subset) and sel1 (exact h1
      subset, 1012 rows) are known, plus over0 = radius union over ALL sel1
      centers, a guaranteed superset (~375 rows) of the final h0 selection.
  Launch B (uniform program "R rows x one param set"): a load-balanced core
      split scores h1[sel1] with level-2 params on most cores while the rest
      run h0[over0] with level-3 params, also emitting the relu hidden x for
      pooling.
  Host: final top-k -> sel0 (subset of over0), softmax pooling, classifier.
"""

import numpy as np

EMBED = 1024
H1 = 512
H2 = 256
TOPP = 0.3
WIN = 225.0
SF = 4.0
OFF = 84.0  # 224/2 - 224/4/2
R2 = np.float32((WIN * SF) ** 2)  # 810000.0, exact in fp32

N_CORES = 8
GMAX = 512  # matmul moving-operand free dim (one fp32 PSUM bank)

# launch A: per-core row counts for (h3, h2) -- 1000 and 4000 divide by 8
# exactly, so shards carry no pad rows
A_LEVEL_ROWS = [1000, 4000]
A_CORE_ROWS = [125, 500]

# Populated on every kernel() call; read by test harnesses.
LAST_RUN_INFO = {}

_CACHE = {}


# ---------------------------------------------------------------- device code


def _emit_level(nc, tc, pools, hT_dram, g_total, w_dram, out_a, out_xT=None, gmax=GMAX,
                wab_early=False, w1_pieces=1):
    """Gated-attention scoring over g_total rows for one parameter set.

    hT_dram: [EMBED, g_total] fp32 (features x rows, this core's shard)
    out_a:   [1, g_total] fp32 -- (a*g)@Wc scores (bias bc added on host)
    out_xT:  optional [128, 4, g_total] fp32, slot (p, m, g) = x[g, m*128+p]
    """
    import concourse.mybir as mybir

    F = mybir.ActivationFunctionType
    ALU = mybir.AluOpType
    wpool, hpool, xpool, agpool, apool, ps, psA = pools

    KC1 = EMBED // 128  # 8 contraction chunks for W1
    M1 = H1 // 128      # 4 output tiles of x
    KC2 = H1 // 128     # 4 contraction chunks for Wa/Wb
    M2 = H2 // 128      # 2 output tiles of a/g
    KC3 = H2 // 128     # 2 contraction chunks for Wc

    dt = mybir.dt.float32

    # ---- weights into SBUF.  DMA issue is serialized (~1.2us fixed per
    # dma_start on SP.SEQ + HWDGE in the cost model), so order by when the
    # compute needs the data (BP + W1-chunk0 + group0-hT-chunk0 unblock the
    # first matmul+relu) and batch everything else into few large DMAs.
    # BP packs [b1(4) | ba(2) | bb(2) | Wc(2)] as 10 per-partition columns;
    # WAB packs [Wa | Wb] along the output-feature axis.
    groups = [
        (g0, min(gmax, g_total - g0)) for g0 in range(0, g_total, gmax)
    ]
    g00, G0 = groups[0]
    # W1BP layout: [bp(10 cols) | chunk0(512) | ... | chunk7(512)] so the
    # first DMA delivers the biases AND W1 chunk 0 in one transfer.
    NB = 10
    wpb = wpool.tile([128, NB + KC1 * H1], dt, tag="w1", name="wpb")
    wpb_src = w_dram["W1BP"]
    hT0 = hpool.tile([128, KC1, gmax], dt, tag="hT", name="hT")[:, :, :G0]
    wab = wpool.tile([128, KC2, 2 * H2], dt, tag="wab", name="wab")
    wab_src = w_dram["WAB"].rearrange("(c p) m -> p c m", p=128)
    nc.sync.dma_start(wpb[:, 0 : NB + H1], wpb_src[:, 0 : NB + H1])
    nc.sync.dma_start(hT0[:, 0, :], hT_dram[0:128, g00 : g00 + G0])
    bounds = [1] + [1 + (KC1 - 1) * (i + 1) // w1_pieces for i in range(w1_pieces)]
    for i in range(w1_pieces):
        lo, hi = bounds[i], bounds[i + 1]
        if lo < hi:
            nc.sync.dma_start(
                wpb[:, NB + lo * H1 : NB + hi * H1],
                wpb_src[:, NB + lo * H1 : NB + hi * H1],
            )

    def w1sl(c, m):
        return wpb[:, NB + c * H1 + m * 128 : NB + c * H1 + (m + 1) * 128]
    nc.sync.dma_start(
        hT0[:, 1:, :],
        hT_dram[128:EMBED, g00 : g00 + G0].rearrange("(c p) g -> p c g", p=128),
    )
    if wab_early:
        nc.sync.dma_start(wab[:], wab_src[:])
    hts = [hT0]
    for g0, G in groups[1:]:
        hTn = hpool.tile([128, KC1, gmax], dt, tag="hT", name="hT")[:, :, :G]
        nc.sync.dma_start(
            hTn[:, :, :],
            hT_dram[:, g0 : g0 + G].rearrange("(c p) g -> p c g", p=128),
        )
        hts.append(hTn)
    if not wab_early:
        nc.sync.dma_start(wab[:], wab_src[:])
    b1 = wpb[:, 0:M1]
    ba = wpb[:, M1 : M1 + M2]
    bb = wpb[:, M1 + M2 : M1 + 2 * M2]
    wc = wpb[:, M1 + 2 * M2 : M1 + 2 * M2 + KC3]

    for gi, (g0, G) in enumerate(groups):
        hT = hts[gi]

        # x^T = relu(W1^T h^T + b1): 4 tiles of [128, G].  relu on DVE keeps
        # ACT on a single table set (tanh+sigmoid).
        xT = xpool.tile([128, M1, gmax], dt, tag="xT", name="xT")[:, :, :G]
        for m in range(M1):
            acc = ps.tile([128, gmax], dt, tag="ps", name="ps")[:, :G]
            for c in range(KC1):
                nc.tensor.matmul(
                    acc,
                    w1sl(c, m),
                    hT[:, c, :],
                    start=(c == 0),
                    stop=(c == KC1 - 1),
                )
            nc.vector.tensor_scalar(
                xT[:, m, :], acc, b1[:, m : m + 1], 0.0, op0=ALU.add, op1=ALU.max
            )

        # a^T = tanh(Wa^T x^T + ba), g^T = sigmoid(Wb^T x^T + bb), ag = a*g
        agT = agpool.tile([128, M2, gmax], dt, tag="agT", name="agT")[:, :, :G]
        for m in range(M2):
            acc_a = ps.tile([128, gmax], dt, tag="ps", name="ps")[:, :G]
            for c in range(KC2):
                nc.tensor.matmul(
                    acc_a,
                    wab[:, c, m * 128 : (m + 1) * 128],
                    xT[:, c, :],
                    start=(c == 0),
                    stop=(c == KC2 - 1),
                )
            aT = agpool.tile([128, gmax], dt, tag="aT", name="aT")[:, :G]
            nc.scalar.activation(aT, acc_a, F.Tanh, bias=ba[:, m : m + 1])

            acc_g = ps.tile([128, gmax], dt, tag="ps", name="ps")[:, :G]
            for c in range(KC2):
                nc.tensor.matmul(
                    acc_g,
                    wab[:, c, 2 * H2 // 2 + m * 128 : 2 * H2 // 2 + (m + 1) * 128],
                    xT[:, c, :],
                    start=(c == 0),
                    stop=(c == KC2 - 1),
                )
            gT = agpool.tile([128, gmax], dt, tag="gT", name="gT")[:, :G]
            nc.scalar.activation(gT, acc_g, F.Sigmoid, bias=bb[:, m : m + 1])
            nc.vector.tensor_mul(agT[:, m, :], aT, gT)

        # A = Wc-contraction of (a*g) -> [1, G]
        acc_s = psA.tile([1, gmax], dt, tag="psA", name="psA")[:, :G]
        for c in range(KC3):
            nc.tensor.matmul(
                acc_s,
                wc[:, c : c + 1],
                agT[:, c, :],
                start=(c == 0),
                stop=(c == KC3 - 1),
            )
        a_st = apool.tile([1, gmax], dt, tag="a_st", name="a_st")[:, :G]
        nc.vector.tensor_copy(a_st, acc_s)
        nc.sync.dma_start(out_a[0:1, g0 : g0 + G], a_st)

        if out_xT is not None:
            nc.sync.dma_start(out_xT[gi, :, :, :G], xT)


def _level_dram_params(nc, tag):
    import concourse.mybir as mybir

    dt = mybir.dt.float32
    return {
        "W1BP": nc.dram_tensor(
            f"W1BP_{tag}", [128, 10 + (EMBED // 128) * H1], dt, kind="ExternalInput"
        ),
        "WAB": nc.dram_tensor(f"WAB_{tag}", [H1, 2 * H2], dt, kind="ExternalInput"),
    }


def _make_pools(tc, ctx):
    wpool = ctx.enter_context(tc.tile_pool(name="weights", bufs=2))
    hpool = ctx.enter_context(tc.tile_pool(name="hin", bufs=3))
    xpool = ctx.enter_context(tc.tile_pool(name="x", bufs=2))
    agpool = ctx.enter_context(tc.tile_pool(name="ag", bufs=2))
    apool = ctx.enter_context(tc.tile_pool(name="aout", bufs=3))
    ps = ctx.enter_context(tc.tile_pool(name="ps", bufs=6, space="PSUM"))
    psA = ctx.enter_context(tc.tile_pool(name="psA", bufs=2, space="PSUM"))
    return wpool, hpool, xpool, agpool, apool, ps, psA


def _build_stage_a():
    import concourse.bacc as bacc
    import concourse.tile as tile
    import concourse.mybir as mybir
    from contextlib import ExitStack

    dt = mybir.dt.float32
    nc = bacc.Bacc(None, target_bir_lowering=False)
    hT_drams = [
        nc.dram_tensor(f"hT{l}", [EMBED, A_CORE_ROWS[l]], dt, kind="ExternalInput")
        for l in range(2)
    ]
    w_drams = [_level_dram_params(nc, str(l)) for l in range(2)]
    out_drams = [
        nc.dram_tensor(f"a{l}", [1, A_CORE_ROWS[l]], dt, kind="ExternalOutput")
        for l in range(2)
    ]
    with tile.TileContext(nc) as tc:
        with ExitStack() as ctx:
            pools = _make_pools(tc, ctx)
            for l in (1, 0):
                _emit_level(
                    nc, tc, pools, hT_drams[l], A_CORE_ROWS[l], w_drams[l],
                    out_drams[l], gmax=250,
                )
    nc.compile()
    return nc


def _build_stage_b(rows):
    """Uniform program: score `rows` rows against ONE param set, emitting
    scores and the relu hidden xT.  Params/rows differ per core via in_maps."""
    import concourse.bacc as bacc
    import concourse.tile as tile
    import concourse.mybir as mybir
    from contextlib import ExitStack

    dt = mybir.dt.float32
    nc = bacc.Bacc(None, target_bir_lowering=False)
    gmax = min(GMAX, max(32, -(-rows // 3)))
    n_groups = -(-rows // gmax)
    hT_dram = nc.dram_tensor("hT", [EMBED, rows], dt, kind="ExternalInput")
    w_dram = _level_dram_params(nc, "w")
    out_a = nc.dram_tensor("a", [1, rows], dt, kind="ExternalOutput")
    out_xT = nc.dram_tensor(
        "xT", [n_groups, 128, H1 // 128, gmax], dt, kind="ExternalOutput"
    )
    with tile.TileContext(nc) as tc:
        with ExitStack() as ctx:
            pools = _make_pools(tc, ctx)
            # PE warm-up: this launch is short, so without it the PE spends
            # most of the kernel below max p-state.  A burst of matmuls on
            # zeroed SBUF bridges the initial DMA window and completes the
            # ~3us continuous-busy ramp before the real matmuls arrive.
            wp = ctx.enter_context(tc.tile_pool(name="warm", bufs=1))
            wdum = wp.tile([128, 128], dt, name="wdum")
            hdum = wp.tile([128, 256], dt, name="hdum")
            nc.gpsimd.memset(wdum[:], 0.0)
            nc.gpsimd.memset(hdum[:], 0.0)
            pdum = pools[5].tile([128, 256], dt, tag="ps", name="pdum")
            for _ in range(8):
                nc.tensor.matmul(pdum[:], wdum[:], hdum[:], start=True, stop=True)
            _emit_level(
                nc, tc, pools, hT_dram, rows, w_dram, out_a, out_xT=out_xT,
                gmax=gmax, wab_early=True, w1_pieces=3,
            )
    nc.compile()
    return nc


# ------------------------------------------------------------------ host code


def _np32(x):
    a = np.asarray(x)
    return a.astype(np.float32) if a.dtype != np.float32 else a


def _pad_rows(h, rows):
    if h.shape[0] == rows:
        return np.ascontiguousarray(h)
    out = np.zeros((rows, h.shape[1]), dtype=h.dtype)
    out[: h.shape[0]] = h
    return out


def _shard_hT(h, core_rows, n_shards):
    """h [N, EMBED] -> n_shards contiguous [EMBED, core_rows] fp32 shards."""
    hp = _pad_rows(_np32(h), core_rows * n_shards)
    hT = np.ascontiguousarray(hp.T)  # [EMBED, padded]
    return [
        np.ascontiguousarray(hT[:, c * core_rows : (c + 1) * core_rows])
        for c in range(n_shards)
    ]


def _level_param_inputs(tag, lp):
    bp = np.zeros((128, 10), dtype=np.float32)
    bp[:, 0:4] = _np32(lp["b1"]).reshape(4, 128).T
    bp[:, 4:6] = _np32(lp["ba"]).reshape(2, 128).T
    bp[:, 6:8] = _np32(lp["bb"]).reshape(2, 128).T
    bp[:, 8:10] = _np32(lp["Wc"]).reshape(-1).reshape(2, 128).T
    w1 = _np32(lp["W1"])  # [1024, 512]; device slot (p, 10+c*512+m) = W1[c*128+p, m]
    w1bp = np.empty((128, 10 + 4096), dtype=np.float32)
    w1bp[:, :10] = bp
    w1bp[:, 10:] = w1.reshape(8, 128, H1).transpose(1, 0, 2).reshape(128, 4096)
    return {
        f"W1BP_{tag}": w1bp,
        f"WAB_{tag}": np.ascontiguousarray(
            np.concatenate([_np32(lp["Wa"]), _np32(lp["Wb"])], axis=1)
        ),
    }


def _topk_centers(a_scores, c_src, k):
    idx = np.argpartition(-a_scores, k - 1)[:k]
    return (c_src[idx] + np.float32(OFF)) * np.float32(SF)


def _radius_union(centers, c_tgt):
    """Exact fp32 replication of the reference's distance test; ascending idx."""
    d = c_tgt[None, :, :] - centers[:, None, :]
    d2 = d[..., 0] * d[..., 0] + d[..., 1] * d[..., 1]
    return np.nonzero((d2 < R2).any(axis=0))[0]


_NEFF_CACHE_DIR = "/var/tmp/bass_neff_cache"


def _install_neff_cache():
    """Memoize walrus NEFF compiles on disk, keyed by the BIR json hash.
    The backend compile (walrus + birsim) costs minutes per NEFF; the BIR
    emitted for a given build is deterministic, so a fresh process can reuse
    a previously compiled NEFF byte-for-byte."""
    import concourse.bass_utils as bu
    import concourse.bass2jax as b2j

    if getattr(bu, "_ant_neff_cache_installed", False):
        return
    import hashlib
    import os
    import shutil

    orig = bu.compile_bir_kernel

    def cached(bir_json, tmpdir, neff_name="file.neff"):
        try:
            os.makedirs(_NEFF_CACHE_DIR, exist_ok=True)
            raw = bir_json if isinstance(bir_json, bytes) else bir_json.encode()
            h = hashlib.sha256(raw).hexdigest()
            cpath = os.path.join(_NEFF_CACHE_DIR, f"{h}.neff")
            out = os.path.join(tmpdir, neff_name)
            if os.path.exists(cpath):
                shutil.copyfile(cpath, out)
                return out
        except Exception:
            return orig(bir_json, tmpdir, neff_name)
        res = orig(bir_json, tmpdir, neff_name)
        try:
            shutil.copyfile(res, cpath + f".tmp{os.getpid()}")
            os.replace(cpath + f".tmp{os.getpid()}", cpath)
        except Exception:
            pass
        return res

    bu.compile_bir_kernel = cached
    b2j.compile_bir_kernel = cached
    bu._ant_neff_cache_installed = True


def _run_spmd(nc, in_maps):
    import jax
    from concourse.bass_utils import run_bass_kernel_spmd

    _install_neff_cache()
    try:
        if not jax.config.jax_compilation_cache_dir:
            jax.config.update("jax_compilation_cache_dir", "/var/tmp/jax_pjrt_cache")
            jax.config.update("jax_persistent_cache_min_compile_time_secs", 0.0)
            jax.config.update("jax_persistent_cache_min_entry_size_bytes", 0)
    except Exception:
        pass
    return run_bass_kernel_spmd(nc, in_maps, core_ids=list(range(N_CORES)))


def kernel(h0, h1, h2, h3, coords0, coords1, coords2, coords3, params):
    h3, h2, h1, h0 = _np32(h3), _np32(h2), _np32(h1), _np32(h0)
    c3, c2, c1, c0 = map(_np32, (coords3, coords2, coords1, coords0))
    levels = params["levels"]

    # ---------------- launch A: scores for all rows of h3, h2
    if "A" not in _CACHE:
        _CACHE["A"] = _build_stage_a()
    nc_a = _CACHE["A"]

    shards = [
        _shard_hT(h3, A_CORE_ROWS[0], N_CORES),
        _shard_hT(h2, A_CORE_ROWS[1], N_CORES),
    ]
    wmaps = {}
    for l in range(2):
        wmaps.update(_level_param_inputs(str(l), levels[l]))
    in_maps = []
    for c in range(N_CORES):
        m = {f"hT{l}": shards[l][c] for l in range(2)}
        m.update(wmaps)
        in_maps.append(m)
    res_a = _run_spmd(nc_a, in_maps)
    A3 = np.concatenate([res_a.results[c]["a0"][0] for c in range(N_CORES)])[
        : A_LEVEL_ROWS[0]
    ]
    A2 = np.concatenate([res_a.results[c]["a1"][0] for c in range(N_CORES)])[
        : A_LEVEL_ROWS[1]
    ]

    # ---------------- host: selection through level 1 + over-approx for h0
    k0 = max(1, int(TOPP * A3.shape[0]))
    sel2 = _radius_union(_topk_centers(A3, c3, k0), c2)

    A2s = A2[sel2]
    k1 = max(1, int(TOPP * A2s.shape[0]))
    sel1 = _radius_union(_topk_centers(A2s, c2[sel2], k1), c1)

    # superset of the final h0 selection: union over ALL sel1 centers
    over0 = _radius_union((c1[sel1] + np.float32(OFF)) * np.float32(SF), c0)

    # ---------------- launch B: h1[sel1] on cores 0..c1n-1, h0[over0] on rest
    n1, n0 = max(1, int(sel1.shape[0])), max(1, int(over0.shape[0]))
    c1n = min(N_CORES - 1, max(1, round(N_CORES * n1 / (n1 + n0))))
    c0n = N_CORES - c1n
    rows = max(-(-n1 // c1n), -(-n0 // c0n))
    key = ("B", rows)
    if key not in _CACHE:
        _CACHE[key] = _build_stage_b(rows)
    nc_b = _CACHE[key]

    sh1 = _shard_hT(h1[sel1], rows, c1n)
    sh0 = _shard_hT(h0[over0], rows, c0n)
    p2 = _level_param_inputs("w", levels[2])
    p3 = _level_param_inputs("w", levels[3])
    in_maps_b = []
    for c in range(N_CORES):
        if c < c1n:
            m = {"hT": sh1[c]}
            m.update(p2)
        else:
            m = {"hT": sh0[c - c1n]}
            m.update(p3)
        in_maps_b.append(m)
    res_b = _run_spmd(nc_b, in_maps_b)

    A1s = np.concatenate([res_b.results[c]["a"][0] for c in range(c1n)])[:n1]
    A0o = np.concatenate([res_b.results[c1n + c]["a"][0] for c in range(c0n)])[:n0]
    def _decode_xT(arr):
        # arr [n_groups, 128, 4, gmax]; row g of group gi -> x[gi*gmax+g, m*128+p]
        ng, _, m4, gm = arr.shape
        x = arr.transpose(0, 3, 2, 1).reshape(ng * gm, m4 * 128)
        return x[:rows]

    x0o = np.concatenate(
        [_decode_xT(res_b.results[c1n + c]["xT"]) for c in range(c0n)]
    )[:n0]

    # ---------------- host: final selection (sel0 is a subset of over0)
    k2 = max(1, int(TOPP * A1s.shape[0]))
    sel0 = _radius_union(_topk_centers(A1s, c1[sel1], k2), c0)
    pos = np.searchsorted(over0, sel0)
    assert np.array_equal(over0[pos], sel0), "over0 must be a superset of sel0"

    bc3 = _np32(levels[3]["bc"]).reshape(-1)[0]
    A_sel = A0o[pos] + bc3
    x = x0o[pos]

    # ---------------- host: softmax pooling + classifier
    am = A_sel.max()
    e = np.exp(A_sel - am)
    w = e / e.sum()
    M = w @ x  # [512]
    logits = (M @ _np32(params["Wcls"]) + _np32(params["bcls"]))[None, :]
    lm = logits.max(axis=1, keepdims=True)
    le = np.exp(logits - lm)
    Y_prob = le / le.sum(axis=1, keepdims=True)
    Y_hat = np.argmax(logits, axis=1).astype(np.int32)
    A_raw = A_sel[None, :].astype(np.float32)

    LAST_RUN_INFO.clear()
    LAST_RUN_INFO.update(
        {
            "n_final": int(sel0.shape[0]),
            "rows_b": int(rows),
            "split": (c1n, c0n),
            "exec_time_ns": [res_a.exec_time_ns, res_b.exec_time_ns],
            "nc_a": nc_a,
            "nc_b": nc_b,
        }
    )
    return (
        logits.astype(np.float32),
        Y_prob.astype(np.float32),
        Y_hat,
        A_raw,
    )
